# revision 4
# baseline (speedup 1.0000x reference)
"""BTT forward kernel v2 for 8 TRN2 NeuronCores.

out[b, y*64+i] = sum_{j,x,r} x[b, j*64+x] * c0[j,x,i,r] * c1[j,y,i,r]
B=8192, D=64, R=4.  Data-parallel over batch (1024 rows/core).

Key improvements over v1 (measured on this hardware):
  - PE streams at 1 cyc/row only when the contraction K=128 partitions;
    K=64 runs at 2 cyc/row.  Stage 1 is therefore packed: j-pairs share
    one matmul with a block-diagonal moving operand (zeros embedded), so
    K=(j-parity, x)=128.  Halves stage-1 PE time.
  - The b<->(j,r) partition swap of z is split: the latency-critical
    low-i quarter of the h1 slab goes to the DMA XBAR transpose (fires
    mid-slot, short enough not to monopolize the DMA queues), the rest
    via PE transpose chains (pure chains run at 1 cyc/row).
  - Output is evacuated to bf16 and DMA'd as bf16 (host upcasts + bias).
  - xt-in and out DMAs issue from the otherwise idle GPSIMD SWDGE queue;
    SP issues the XBARs.
  - PSUM->SBUF evacuations are assigned to DVE/ACT by a running cost
    model (DVE reads bf16 PSUM ~0.83 ns/col vs ~1.35 fp32; ACT ~1.35
    for both).

Per core, per 128-row batch tile bt:
  s1   : 16 pz tiles [128b, 1024] fp32 (2 banks, 2 packed matmuls each,
         K=128, F=512, 216ns) -> evac to z_h[b, i*128 + j32*4 + r] bf16
  trans: h0: 64 PE transposes in 8 chains -> z2_h0[(j32,r), i*128+b]
         h1: one XBAR dma  z_h1 [128, 8192] -> z2_h1 [128, 64, 128]
  s2   : 128 matmuls (c1 stationary [128,64], K=(j32,r) halves
         accumulated, F=128, 55ns) -> po [128=(par,y), 512] fp32
         -> evac bf16 -> out DMA (gpsimd)
"""

import numpy as np

import concourse.bass as bass
import concourse.mybir as mybir
from concourse.bass_utils import run_bass_kernel_spmd
from concourse.tile import TileContext

N_CORES = 8
TRACE = False
LAST_RESULT = None
B = 8192
D = 64
R = 4
BC = B // N_CORES          # 1024 rows per core
BT = 128                   # batch tile
NBT = BC // BT             # 8

BF16 = mybir.dt.bfloat16
F32 = mybir.dt.float32


def _split_multi_waits(nc: bass.Bass):
    """This container's walrus accepts only ONE sync-wait per instruction."""
    fn = nc.m.functions[0]
    for bb in fn.blocks:
        new_insts = []
        changed = False
        for ins in bb.instructions:
            si = ins.sync_info
            if si is not None and si.on_wait and len(si.on_wait) > 1:
                changed = True
                waits = list(si.on_wait)
                del si.on_wait[:]
                si.on_wait.append(waits[-1])
                for k, w in enumerate(waits[:-1]):
                    nop = mybir.InstNoOp(
                        name=f"{ins.name}-wsplit{k}",
                        sync_info=mybir.SyncInfo(on_wait=[w], on_update=[]),
                        bass_nofuse=True,
                        engine=ins.engine,
                    )
                    nc.register_instruction(nop)
                    new_insts.append(nop)
            new_insts.append(ins)
        if changed:
            bb.instructions = new_insts


def build_nc() -> bass.Bass:
    nc = bass.Bass()

    xt_d = nc.declare_dram_parameter("xt", [NBT, 128, 32 * BT], BF16,
                                     isOutput=False)
    c0p_d = nc.declare_dram_parameter("c0p", [128, 32 * 512], BF16,
                                      isOutput=False)
    c1_d = nc.declare_dram_parameter("c1", [2, 128, D * D], BF16,
                                     isOutput=False)
    id_d = nc.declare_dram_parameter("ident", [128, 128], BF16,
                                     isOutput=False)
    out_d = nc.declare_dram_parameter("out", [NBT, 128, D * D], BF16,
                                      isOutput=True)

    # running engine-load estimate for evac assignment (ns)
    load = {"v": 0.0, "s": 0.0}
    RATE = {("v", "f32"): 1.35, ("v", "bf16"): 0.83,
            ("s", "f32"): 1.35, ("s", "bf16"): 1.35}

    def evac(dst_ap, src_ap, kind, cols):
        cv = load["v"] + cols * RATE[("v", kind)]
        cs = load["s"] + cols * RATE[("s", kind)]
        if cv <= cs:
            load["v"] = cv + 170
            nc.vector.tensor_copy(dst_ap, src_ap)
        else:
            load["s"] = cs + 170
            nc.scalar.copy(dst_ap, src_ap)

    with TileContext(nc) as tc:
        with (
            tc.tile_pool(name="const", bufs=1) as cpool,
            tc.tile_pool(name="xt", bufs=2) as xpool,
            tc.tile_pool(name="z0", bufs=1) as z0pool,
            tc.tile_pool(name="z1", bufs=2) as z1pool,
            tc.tile_pool(name="z2", bufs=2) as z2pool,
            tc.tile_pool(name="osb", bufs=8) as opool,
            tc.tile_pool(name="psz", bufs=4, space="PSUM") as pszpool,
            tc.tile_pool(name="pst", bufs=2, space="PSUM") as ptpool,
            tc.tile_pool(name="pso", bufs=2, space="PSUM") as popool,
        ):
            # load order: ident first (tiny, enables PE warmup), then xt0 +
            # the h1 half of c0p (first data s1 needs), then the rest; c1
            # last (first needed by s2, one slot later).
            ident = cpool.tile([128, 128], BF16, tag="ident")
            nc.sync.dma_start(ident[:], id_d[:])
            c0p = cpool.tile([128, 32 * 512], BF16, tag="c0p")
            c1_sb = [
                cpool.tile([128, D * D], BF16, tag=f"c1_{h}", name=f"c1_{h}")
                for h in (0, 1)
            ]

            def prefetch_xt(bt):
                t = xpool.tile([128, 32 * BT], BF16, tag="xt", name="xt")
                nc.sync.dma_start(t[:], xt_d[bt])
                xt_of[bt] = t

            def s1_emitter(bt):
                """Packed stage 1 (h1 half first) + h1 XBAR + h0 PE chains."""
                if bt + 1 < NBT:
                    prefetch_xt(bt + 1)
                xt = xt_of.pop(bt)
                z_h = [
                    z0pool.tile([BT, 64 * BT], BF16, tag="z0", name="z0"),
                    z1pool.tile([BT, 64 * BT], BF16, tag="z1", name="z1"),
                ]
                zh_of[bt] = z_h
                z2_h = [
                    z2pool.tile([128, 64 * BT], BF16, tag="z20", name="z20"),
                    z2pool.tile([128, 64 * BT], BF16, tag="z21", name="z21"),
                ]
                z2_of[bt] = z2_h
                # h1 half (jj 8..15) first so the XBAR can launch mid-slot
                for jj in list(range(8, 16)) + list(range(8)):
                    h = jj // 8
                    for l in (0, 1):
                        jp = 2 * jj + l
                        pz = pszpool.tile([BT, 512], F32, tag="pz",
                                          name="pz")
                        nc.tensor.matmul(
                            pz[:],
                            xt[:, jp * BT:(jp + 1) * BT],
                            c0p[:, jp * 512:(jp + 1) * 512],
                        )
                        # pz cols (par, i, r) -> z col 128*i + j32*4 + r,
                        # j32 = 4*(jj%8) + 2*l + par
                        dst = z_h[h][:].rearrange(
                            "p (i jq lp r) -> p jq lp i r",
                            i=64, jq=8, lp=4, r=4
                        )[:, jj % 8, 2 * l:2 * l + 2]
                        src = pz[:].rearrange(
                            "p (pr i r) -> p pr i r", pr=2, i=64, r=4
                        )
                        evac(dst, src, "f32", 512)
                    yield
                    if jj == 15:
                        # h1 complete: the latency-critical low-i span via
                        # a short XBAR on SP (the first s2 ogs wait on it);
                        # the high-i tail via PE chains right away
                        nc.sync.dma_start(
                            z2_h[1][:, 0:5120].rearrange(
                                "p (i b) -> p i b", i=40, b=BT),
                            z_h[1][:, 0:5120],
                            transpose=True,
                        )
                        for c8 in (5, 6, 7):
                            yield from pe_chain(z_h[1], z2_h[1], c8)
                # h0 complete: PE transpose chains
                for c8 in range(8):
                    yield from pe_chain(z_h[0], z2_h[0], c8)
                return

            def pe_chain(z_t, z2_t, c8):
                pt = ptpool.tile([128, 1024], BF16, tag="pt", name="pt")
                for ii in range(8):
                    i = c8 * 8 + ii
                    nc.tensor.matmul(
                        pt[:, ii * BT:(ii + 1) * BT],
                        z_t[:, i * BT:(i + 1) * BT],
                        ident[:],
                        is_transpose=True,
                        start=(ii == 0),
                        stop=(ii == 7),
                    )
                evac(z2_t[:, c8 * 1024:(c8 + 1) * 1024], pt[:], "bf16", 1024)
                yield

            def s2_emitter(bt, z2_h):
                for og in range(8):
                    po = popool.tile([BT, 512], F32, tag="po", name="po")
                    for h in (0, 1):
                        for q4 in range(4):
                            ipair = og * 4 + q4
                            for par in (0, 1):
                                i = 2 * ipair + par
                                nc.tensor.matmul(
                                    po[par * 64:(par + 1) * 64,
                                       q4 * BT:(q4 + 1) * BT],
                                    c1_sb[h][:, i * D:(i + 1) * D],
                                    z2_h[h][:, i * BT:(i + 1) * BT],
                                    start=(q4 == 0 and h == 0),
                                    stop=(q4 == 3 and h == 1),
                                    skip_group_check=(par == 1),
                                )
                    osb = opool.tile([BT, 512], BF16, tag="osb", name="osb")
                    evac(osb[:], po[:], "f32", 512)
                    eng = nc.sync if bt == NBT - 1 else nc.gpsimd
                    eng.dma_start(
                        out_d[bt][:, og * 512:(og + 1) * 512], osb[:]
                    )
                    yield
                return

            zh_of = {}
            z2_of = {}
            xt_of = {}
            prefetch_xt(0)
            for q0 in (2, 3, 0, 1):
                nc.sync.dma_start(
                    c0p[:, q0 * 4096:(q0 + 1) * 4096],
                    c0p_d[:, q0 * 4096:(q0 + 1) * 4096],
                )
            for h in (0, 1):
                nc.sync.dma_start(c1_sb[h][:], c1_d[h])
            # PE warmup: ramp the clock while the input DMAs stream
            wt = ptpool.tile([128, 1024], BF16, tag="pt", name="ptwarm")
            for w in range(120):
                nc.tensor.matmul(
                    wt[:, (w % 8) * 128:(w % 8) * 128 + 128],
                    ident[:], ident[:],
                    is_transpose=True,
                    start=(w % 8 == 0), stop=(w % 8 == 7),
                    skip_group_check=True,
                )
            prev = None
            for bt in range(NBT + 1):
                cur = s1_emitter(bt) if bt < NBT else None
                gens = []
                if cur is not None:
                    gens.append(("s1", cur))
                if prev is not None:
                    gens.append(("s2", prev))
                alive = dict(gens)
                while alive:
                    for key, ratio in (("s1", 3), ("s2", 1)):
                        g = alive.get(key)
                        if g is None:
                            continue
                        for _ in range(ratio):
                            try:
                                next(g)
                            except StopIteration:
                                del alive[key]
                                break
                if bt < NBT:
                    prev = s2_emitter(bt, z2_of.pop(bt))
                else:
                    prev = None

    _split_multi_waits(nc)
    return nc


def _host_prep(x):
    """Full-model host-side input prep; returns per-core in_maps."""
    import ml_dtypes

    bf = ml_dtypes.bfloat16
    return x.astype(bf)


def kernel(x, core0, core1, bias):
    import ml_dtypes

    bf = ml_dtypes.bfloat16
    x = np.asarray(x, np.float32)
    c0 = np.asarray(core0, np.float32).reshape(D, D, D, R)   # j, x, i, r
    c1 = np.asarray(core1, np.float32).reshape(D, D, D, R)   # j, y, i, r
    bias = np.asarray(bias, np.float32)

    # c0p[par*64+x, jp*512 + par2*256 + i*4 + r] =
    #     (par == par2) * c0[2*jp+par, x, i, r]
    c0p = np.zeros((2, 64, 32, 2, 256), dtype=np.float32)
    c0r = c0.reshape(32, 2, D, D * R)                        # jp, par, x, (i r)
    for par in (0, 1):
        c0p[par, :, :, par, :] = c0r[:, par].transpose(1, 0, 2)
    c0p = c0p.reshape(128, 32 * 512).astype(bf)

    # c1_arr[h, j32*4+r, i*64+y] = c1[h*32+j32, y, i, r]
    c1_arr = (
        c1.reshape(2, 32, D, D, R).transpose(0, 1, 4, 3, 2)
        .reshape(2, 128, D * D).astype(bf)
    )
    ident = np.eye(128, dtype=np.float32).astype(bf)

    nc = build_nc()
    in_maps = []
    for c in range(N_CORES):
        xc = x[c * BC:(c + 1) * BC]
        # xt[bt, par*64+x, jp*128+b] = xc[bt*128+b, (2*jp+par)*64 + x]
        xt = (
            xc.reshape(NBT, BT, 32, 2, 64)       # bt, b, jp, par, x
            .transpose(0, 3, 4, 2, 1)             # bt, par, x, jp, b
            .reshape(NBT, 128, 32 * BT)
            .astype(bf)
        )
        in_maps.append({"xt": xt, "c0p": c0p, "c1": c1_arr, "ident": ident})

    res = run_bass_kernel_spmd(
        nc, in_maps, core_ids=list(range(N_CORES)), trace=TRACE
    )
    global LAST_RESULT
    LAST_RESULT = res

    # out_dev[bt, par*64+y, ipair*128+b] -> out[bt*128+b, y*64+2*ipair+par]
    outs = []
    for c in range(N_CORES):
        od = np.asarray(res.results[c]["out"], np.float32)
        od = od.reshape(NBT, 2, D, 32, BT)       # bt, par, y, ipair, b
        oc = od.transpose(0, 4, 2, 3, 1).reshape(BC, D * D)
        outs.append(oc)
    out = np.concatenate(outs, axis=0)
    return (out + bias[None, :]).astype(np.float32)
